# revision 7
# baseline (speedup 1.0000x reference)
# Trainium2 Bass kernel for nn_Connection_geognn_17076789969601.
#
# Math (per sample row of input_ [N, 128], x = row[:64], v = row[64:]):
#   h  = tanh(W1 @ x + b1)                  # [128]
#   Wm = tanh(W2 @ h + b2).reshape(64, 4)   # [64, 4]
#   u  = v @ Wm ;  H = sum(u^2)
#   output = [dH/dx, -dH/dv]
#
# Backward (per sample), with T = tanh(A2) in W2-row-permuted order so that
# column j of Wm occupies rows [64j, 64j+64):
#   dv_out = -2 Wm u
#   dA2    = 2 (v x u) * (1 - T^2) = Q - M,  Q = 2u*v,  M = 2u*v*T^2
#   dh     = W2r^T dA2 ;  dA1 = (1-h^2)*dh ;  dx = W1^T dA1
#
# Device layout: feature-major streams [feat(part), samples(free)], B=512
# samples per macro tile.  The two 128-row halves of the T-space are
# "f-stacked" along the free axis into single [128, 2B] tiles (cols [0,B) =
# half a, [B,2B) = half b), so each elementwise stage is ONE wide instruction
# (FD=1024 hits the DVE 2x bf16 mode; per-instruction overhead amortized).
# The per-half b2 bias is folded into the A2 accumulation as a rank-1 matmul
# (b2 x ones-row).  v-replication uses stride-0 broadcast APs (no vrep2
# materialization).  Input/output are transposed ON HOST so all DMA is
# contiguous.  u's block-sum+broadcast (mblk) and the dv pair-sum (msum) are
# constant mask matmuls; dA2 is never materialized - Q and M feed sign-folded
# accumulating matmuls for dh.
#
# Engine placement per B-tile: Act {tanh x2, R-copy, h1sq}; DVE {P, S, Q,
# dA1-STT, out-copy}; Pool {M}; PE {14 matmuls}.
# PSUM: a1(1) + a2(2) + r(2) + dh1(1) + outq(1x2bufs) = 8 banks.
#
# Sharding: pure data parallel over 8 NeuronCores, batch 262144 -> 8 x 32768,
# weights replicated.

import sys

sys.path.insert(0, "/opt/trn_rl_repo")

import numpy as np
import ml_dtypes

import concourse.bass as bass
import concourse.bacc as bacc
import concourse.tile as tile
import concourse.mybir as mybir
from concourse.bass_utils import run_bass_kernel_spmd

F32 = mybir.dt.float32
BF16 = mybir.dt.bfloat16
AF = mybir.ActivationFunctionType
ALU = mybir.AluOpType

D = 64
RANK = 4
N_TOTAL = 262144
N_CORES = 8
N_ROWS = N_TOTAL // N_CORES  # 32768 per core
B = 512                      # samples per macro tile


def _half2(ap, b):
    """[128, 2b] AP -> [128, 2, b] (half index as middle dim)."""
    return ap.rearrange("p (t f) -> p t f", t=2)


def build_program(n_rows=N_ROWS, b=B):
    nt = n_rows // b
    nc = bacc.Bacc()

    inp = nc.declare_dram_parameter("inp", [128, n_rows], BF16, isOutput=False)
    w1t = nc.declare_dram_parameter("w1t", [64, 128], BF16, isOutput=False)
    w2ta = nc.declare_dram_parameter("w2ta", [128, 128], BF16, isOutput=False)
    w2tb = nc.declare_dram_parameter("w2tb", [128, 128], BF16, isOutput=False)
    w2pa = nc.declare_dram_parameter("w2pa", [128, 128], BF16, isOutput=False)
    w2pb = nc.declare_dram_parameter("w2pb", [128, 128], BF16, isOutput=False)
    w2ma = nc.declare_dram_parameter("w2ma", [128, 128], BF16, isOutput=False)
    w2mb = nc.declare_dram_parameter("w2mb", [128, 128], BF16, isOutput=False)
    w1n = nc.declare_dram_parameter("w1n", [128, 64], BF16, isOutput=False)
    mblk = nc.declare_dram_parameter("mblk", [128, 128], BF16, isOutput=False)
    msum = nc.declare_dram_parameter("msum", [128, 64], BF16, isOutput=False)
    b2al = nc.declare_dram_parameter("b2al", [1, 128], BF16, isOutput=False)
    b2bl = nc.declare_dram_parameter("b2bl", [1, 128], BF16, isOutput=False)
    ones = nc.declare_dram_parameter("ones", [1, 512], BF16, isOutput=False)
    b1p = nc.declare_dram_parameter("b1", [128, 1], F32, isOutput=False)
    outp = nc.declare_dram_parameter("out", [128, n_rows], BF16, isOutput=True)

    with tile.TileContext(nc) as tc:
        with (
            tc.tile_pool(name="const", bufs=1) as cp,
            tc.tile_pool(name="sb", bufs=4) as sb,
            tc.tile_pool(name="ps", bufs=1, space="PSUM") as ps,
        ):
            c_w1t = cp.tile([64, 128], BF16, tag="w1t")
            c_w2ta = cp.tile([128, 128], BF16, tag="w2ta")
            c_w2tb = cp.tile([128, 128], BF16, tag="w2tb")
            c_w2pa = cp.tile([128, 128], BF16, tag="w2pa")
            c_w2pb = cp.tile([128, 128], BF16, tag="w2pb")
            c_w2ma = cp.tile([128, 128], BF16, tag="w2ma")
            c_w2mb = cp.tile([128, 128], BF16, tag="w2mb")
            c_w1n = cp.tile([128, 64], BF16, tag="w1n")
            c_mblk = cp.tile([128, 128], BF16, tag="mblk")
            c_msum = cp.tile([128, 64], BF16, tag="msum")
            c_b2al = cp.tile([1, 128], BF16, tag="b2al")
            c_b2bl = cp.tile([1, 128], BF16, tag="b2bl")
            c_ones = cp.tile([1, 512], BF16, tag="ones")
            c_b1 = cp.tile([128, 1], F32, tag="b1")
            for t_, p_ in (
                (c_w1t, w1t), (c_w2ta, w2ta), (c_w2tb, w2tb),
                (c_w2pa, w2pa), (c_w2pb, w2pb), (c_w2ma, w2ma),
                (c_w2mb, w2mb), (c_w1n, w1n), (c_mblk, mblk),
                (c_msum, msum), (c_b2al, b2al), (c_b2bl, b2bl), (c_ones, ones), (c_b1, b1p),
            ):
                nc.sync.dma_start(t_[:], p_[:])

            for t in range(nt):
                # ---- contiguous input load + v replication (DMA only) ----
                tint = sb.tile([128, b], BF16, tag="INT")   # [x^T; v^T]
                nc.sync.dma_start(tint[:], inp[:, bass.ts(t, b)])
                vtile = sb.tile([128, b], BF16, tag="VT")   # [v^T; v^T]
                nc.sync.dma_start(vtile[0:64, :], tint[64:128, :])
                nc.sync.dma_start(vtile[64:128, :], tint[64:128, :])
                # stride-0 "2 halves" view of vtile for FD-2b ops
                vrep2 = vtile[:].rearrange("p (o f) -> p o f", o=1) \
                                .broadcast_to([128, 2, b])

                # ---- forward layer 1 ----
                a1 = ps.tile([128, b], F32, tag="a1")
                nc.tensor.matmul(a1[:], c_w1t[:], tint[0:64, :],
                                 start=True, stop=True)
                h1 = sb.tile([128, b], BF16, tag="H1")
                nc.scalar.activation(h1[:], a1[:], AF.Tanh, bias=c_b1[:, 0:1])

                # ---- forward layer 2, f-stacked halves; bias via rank-1 mm --
                a2 = ps.tile([128, 2 * b], F32, tag="a2")
                nc.tensor.matmul(a2[:, 0:b], c_b2al[:], c_ones[:, 0:b],
                                 start=True, stop=False)
                nc.tensor.matmul(a2[:, 0:b], c_w2ta[:], h1[:],
                                 start=False, stop=True)
                nc.tensor.matmul(a2[:, b:2 * b], c_b2bl[:], c_ones[:, 0:b],
                                 start=True, stop=False)
                nc.tensor.matmul(a2[:, b:2 * b], c_w2tb[:], h1[:],
                                 start=False, stop=True)
                t2 = sb.tile([128, 2 * b], BF16, tag="T2")
                nc.scalar.activation(t2[:], a2[:], AF.Tanh)

                # ---- P = vrep*T2 ; R = mblk @ P = 2u broadcast (PSUM) ----
                p = sb.tile([128, 2 * b], BF16, tag="P")
                nc.vector.tensor_mul(_half2(p[:], b), vrep2, _half2(t2[:], b))
                r = ps.tile([128, 2 * b], F32, tag="r")
                nc.tensor.matmul(r[:, 0:b], c_mblk[:], p[:, 0:b],
                                 start=True, stop=True)
                nc.tensor.matmul(r[:, b:2 * b], c_mblk[:], p[:, b:2 * b],
                                 start=True, stop=True)
                rc = sb.tile([128, 2 * b], BF16, tag="RC")
                nc.scalar.copy(rc[:], r[:])

                # ---- S = Rc*T2 -> dv (sign folded into msum) ----
                s = sb.tile([128, 2 * b], BF16, tag="S")
                nc.vector.tensor_mul(s[:], rc[:], t2[:])
                outq = ps.tile([128, b], F32, tag="outq", bufs=2)
                nc.tensor.matmul(outq[64:128, :], c_msum[:], s[:, 0:b],
                                 start=True, stop=False)
                nc.tensor.matmul(outq[64:128, :], c_msum[:], s[:, b:2 * b],
                                 start=False, stop=True)

                # ---- Q = Rc*vrep (DVE), M = S*P (Pool) ----
                q = sb.tile([128, 2 * b], BF16, tag="Q")
                nc.vector.tensor_mul(_half2(q[:], b), _half2(rc[:], b), vrep2)
                m = sb.tile([128, 2 * b], BF16, tag="M")
                nc.gpsimd.tensor_mul(m[:], s[:], p[:])

                # ---- dh1 = W2ra^T Qa + W2rb^T Qb - W2ra^T Ma - W2rb^T Mb ----
                dh1 = ps.tile([128, b], F32, tag="dh1")
                nc.tensor.matmul(dh1[:], c_w2pa[:], q[:, 0:b],
                                 start=True, stop=False)
                nc.tensor.matmul(dh1[:], c_w2pb[:], q[:, b:2 * b],
                                 start=False, stop=False)
                nc.tensor.matmul(dh1[:], c_w2ma[:], m[:, 0:b],
                                 start=False, stop=False)
                nc.tensor.matmul(dh1[:], c_w2mb[:], m[:, b:2 * b],
                                 start=False, stop=True)

                # ---- dA1m = (h1^2 - 1)*dh1 ; dx = -W1^T dA1m ----
                h1sq = sb.tile([128, b], BF16, tag="H1sq")
                nc.scalar.activation(h1sq[:], h1[:], AF.Square)
                da1 = sb.tile([128, b], BF16, tag="dA1")
                nc.vector.scalar_tensor_tensor(
                    da1[:], h1sq[:], 1.0, dh1[:], ALU.subtract, ALU.mult)
                nc.tensor.matmul(outq[0:64, :], c_w1n[:], da1[:],
                                 start=True, stop=True)

                # ---- PSUM -> SBUF bf16, contiguous store ----
                outs = sb.tile([128, b], BF16, tag="OUTS")
                nc.vector.tensor_copy(outs[:], outq[:])
                nc.sync.dma_start(outp[:, bass.ts(t, b)], outs[:])

    nc.finalize()
    return nc


def make_consts(W1, b1, W2, b2):
    """Host-side constant preparation (permutes W2 rows, folds signs)."""
    bf = ml_dtypes.bfloat16
    W1 = np.asarray(W1, np.float32)
    b1 = np.asarray(b1, np.float32)
    W2 = np.asarray(W2, np.float32)
    b2 = np.asarray(b2, np.float32)
    perm = np.empty(RANK * D, np.int64)
    for j in range(RANK):
        for i in range(D):
            perm[j * D + i] = i * RANK + j
    W2r = W2[perm, :]
    b2r = b2[perm]
    mblk = np.zeros((128, 128), np.float32)
    mblk[:64, :64] = 2.0
    mblk[64:, 64:] = 2.0
    msum = np.zeros((128, 64), np.float32)
    for i in range(64):
        msum[i, i] = -1.0
        msum[64 + i, i] = -1.0
    return {
        "w1t": np.ascontiguousarray(W1.T).astype(bf),
        "w2ta": np.ascontiguousarray(W2r[:128].T).astype(bf),
        "w2tb": np.ascontiguousarray(W2r[128:].T).astype(bf),
        "w2pa": np.ascontiguousarray(W2r[:128]).astype(bf),
        "w2pb": np.ascontiguousarray(W2r[128:]).astype(bf),
        "w2ma": np.ascontiguousarray(-W2r[:128]).astype(bf),
        "w2mb": np.ascontiguousarray(-W2r[128:]).astype(bf),
        "w1n": np.ascontiguousarray(-W1).astype(bf),
        "mblk": mblk.astype(bf),
        "msum": msum.astype(bf),
        "b2al": np.ascontiguousarray(b2r[:128].reshape(1, 128)).astype(bf),
        "b2bl": np.ascontiguousarray(b2r[128:].reshape(1, 128)).astype(bf),
        "ones": np.ones((1, 512), np.float32).astype(bf),
        "b1": b1.reshape(128, 1).astype(np.float32),
    }


_NC_CACHE = {}


def _get_program(n_rows, b):
    key = (n_rows, b)
    if key not in _NC_CACHE:
        _NC_CACHE[key] = build_program(n_rows, b)
    return _NC_CACHE[key]


def make_in_maps(inputs):
    input_ = np.asarray(inputs["input_"], np.float32)
    n = input_.shape[0]
    n_rows = n // N_CORES
    consts = make_consts(inputs["W1"], inputs["b1"], inputs["W2"], inputs["b2"])
    bfl = ml_dtypes.bfloat16
    in_maps = []
    for c in range(N_CORES):
        sh = input_[c * n_rows:(c + 1) * n_rows]          # [n_rows, 128]
        m = {"inp": np.ascontiguousarray(sh.T).astype(bfl)}  # [128, n_rows]
        m.update(consts)
        in_maps.append(m)
    return in_maps


def kernel(t, input_, W1, b1, W2, b2):
    input_ = np.asarray(input_, np.float32)
    n = input_.shape[0]
    n_rows = n // N_CORES
    nc = _get_program(n_rows, B)
    in_maps = make_in_maps(
        {"input_": input_, "W1": W1, "b1": b1, "W2": W2, "b2": b2})
    res = run_bass_kernel_spmd(nc, in_maps, list(range(N_CORES)))
    out = np.concatenate(
        [np.asarray(res.results[c]["out"]).astype(np.float32).T
         for c in range(N_CORES)], axis=0)
    return out


# revision 9
# speedup vs baseline: 1.0513x; 1.0513x over previous
# Trainium2 Bass kernel for nn_Connection_geognn_17076789969601.
#
# Math (per sample row of input_ [N, 128], x = row[:64], v = row[64:]):
#   h  = tanh(W1 @ x + b1)                  # [128]
#   Wm = tanh(W2 @ h + b2).reshape(64, 4)   # [64, 4]
#   u  = v @ Wm ;  H = sum(u^2)
#   output = [dH/dx, -dH/dv]
#
# Backward (per sample), with T = tanh(A2) in W2-row-permuted order so that
# column j of Wm occupies rows [64j, 64j+64):
#   dv_out = -2 Wm u
#   dA2    = 2 (v x u) * (1 - T^2) = Q - M,  Q = 2u*v,  M = 2u*v*T^2
#   dh     = W2r^T dA2 ;  dA1 = (1-h^2)*dh ;  dx = W1^T dA1
#
# Device layout: feature-major streams [feat(part), samples(free)].  The
# PSUM-coupled stages (matmuls, tanh, R-copy, dA1, out-copy) run on B=512
# macro tiles so every PSUM tag fits its own bank (a1,a2a,a2b,ra,rb,dh1 x1 +
# outq x2 = 8 banks, pipelined).  The pure-SBUF elementwise products (P, S,
# Q, M, h1sq) run PAIR-WIDE (FD=1024) on [128, 1024] SBUF tiles shared by two
# consecutive macro tiles - one wide DVE instruction costs ~690ns (2x bf16
# mode) vs 2x635ns for the split version, and M/h1sq ride the GpSimd engine.
# Input/output are transposed ON HOST so all DMA is contiguous.  u's
# block-sum+broadcast (mblk) and the dv pair-sum (msum) are constant mask
# matmuls; dA2 is never materialized - Q and M feed sign-folded accumulating
# matmuls for dh.  TT operand order keeps plain APs in src0.
#
# Sharding: pure data parallel over 8 NeuronCores, batch 262144 -> 8 x 32768,
# weights replicated.

import sys

sys.path.insert(0, "/opt/trn_rl_repo")

import numpy as np
import ml_dtypes

import concourse.bass as bass
import concourse.bacc as bacc
import concourse.tile as tile
import concourse.mybir as mybir
from concourse.bass_utils import run_bass_kernel_spmd

F32 = mybir.dt.float32
BF16 = mybir.dt.bfloat16
AF = mybir.ActivationFunctionType
ALU = mybir.AluOpType

D = 64
RANK = 4
N_TOTAL = 262144
N_CORES = 8
N_ROWS = N_TOTAL // N_CORES  # 32768 per core
B = 512                      # samples per PSUM macro tile (2 per DVE pair)


def build_program(n_rows=N_ROWS, b=B):
    npair = n_rows // (2 * b)
    nc = bacc.Bacc()

    inp = nc.declare_dram_parameter("inp", [128, n_rows], BF16, isOutput=False)
    w1t = nc.declare_dram_parameter("w1t", [64, 128], BF16, isOutput=False)
    w2ta = nc.declare_dram_parameter("w2ta", [128, 128], BF16, isOutput=False)
    w2tb = nc.declare_dram_parameter("w2tb", [128, 128], BF16, isOutput=False)
    w2pa = nc.declare_dram_parameter("w2pa", [128, 128], BF16, isOutput=False)
    w2pb = nc.declare_dram_parameter("w2pb", [128, 128], BF16, isOutput=False)
    w2ma = nc.declare_dram_parameter("w2ma", [128, 128], BF16, isOutput=False)
    w2mb = nc.declare_dram_parameter("w2mb", [128, 128], BF16, isOutput=False)
    w1n = nc.declare_dram_parameter("w1n", [128, 64], BF16, isOutput=False)
    mblk = nc.declare_dram_parameter("mblk", [128, 128], BF16, isOutput=False)
    msum = nc.declare_dram_parameter("msum", [128, 64], BF16, isOutput=False)
    b1p = nc.declare_dram_parameter("b1", [128, 1], F32, isOutput=False)
    b2ap = nc.declare_dram_parameter("b2a", [128, 1], F32, isOutput=False)
    b2bp = nc.declare_dram_parameter("b2b", [128, 1], F32, isOutput=False)
    outp = nc.declare_dram_parameter("out", [128, n_rows], BF16, isOutput=True)

    with tile.TileContext(nc) as tc:
        with (
            tc.tile_pool(name="const", bufs=1) as cp,
            tc.tile_pool(name="sb", bufs=3) as sb,
            tc.tile_pool(name="ps", bufs=1, space="PSUM") as ps,
        ):
            c_w1t = cp.tile([64, 128], BF16, tag="w1t")
            c_w2ta = cp.tile([128, 128], BF16, tag="w2ta")
            c_w2tb = cp.tile([128, 128], BF16, tag="w2tb")
            c_w2pa = cp.tile([128, 128], BF16, tag="w2pa")
            c_w2pb = cp.tile([128, 128], BF16, tag="w2pb")
            c_w2ma = cp.tile([128, 128], BF16, tag="w2ma")
            c_w2mb = cp.tile([128, 128], BF16, tag="w2mb")
            c_w1n = cp.tile([128, 64], BF16, tag="w1n")
            c_mblk = cp.tile([128, 128], BF16, tag="mblk")
            c_msum = cp.tile([128, 64], BF16, tag="msum")
            c_b1 = cp.tile([128, 1], F32, tag="b1")
            c_b2a = cp.tile([128, 1], F32, tag="b2a")
            c_b2b = cp.tile([128, 1], F32, tag="b2b")
            for t_, p_ in (
                (c_w1t, w1t), (c_w2ta, w2ta), (c_w2tb, w2tb),
                (c_w2pa, w2pa), (c_w2pb, w2pb), (c_w2ma, w2ma),
                (c_w2mb, w2mb), (c_w1n, w1n), (c_mblk, mblk),
                (c_msum, msum), (c_b1, b1p), (c_b2a, b2ap), (c_b2b, b2bp),
            ):
                nc.sync.dma_start(t_[:], p_[:])

            for k in range(npair):
                # ---- pair-wide input load + v replication (DMA only) ----
                tint = sb.tile([128, 2 * b], BF16, tag="INT")
                nc.sync.dma_start(tint[:], inp[:, bass.ts(k, 2 * b)])
                vt = sb.tile([128, 2 * b], BF16, tag="VT")
                nc.sync.dma_start(vt[0:64, :], tint[64:128, :])
                nc.sync.dma_start(vt[64:128, :], tint[64:128, :])

                h1 = sb.tile([128, 2 * b], BF16, tag="H1")
                t2a = sb.tile([128, 2 * b], BF16, tag="T2a")
                t2b = sb.tile([128, 2 * b], BF16, tag="T2b")
                rca = sb.tile([128, 2 * b], BF16, tag="RCa")
                rcb = sb.tile([128, 2 * b], BF16, tag="RCb")

                # ---- forward, per 512-half (PSUM tags single-bank) ----
                for i in range(2):
                    hs = bass.ts(i, b)
                    a1 = ps.tile([128, b], F32, tag="a1")
                    nc.tensor.matmul(a1[:], c_w1t[:], tint[0:64, hs],
                                     start=True, stop=True)
                    nc.scalar.activation(h1[:, hs], a1[:], AF.Tanh,
                                         bias=c_b1[:, 0:1])
                    a2a = ps.tile([128, b], F32, tag="a2a")
                    a2b = ps.tile([128, b], F32, tag="a2b")
                    nc.tensor.matmul(a2a[:], c_w2ta[:], h1[:, hs],
                                     start=True, stop=True)
                    nc.tensor.matmul(a2b[:], c_w2tb[:], h1[:, hs],
                                     start=True, stop=True)
                    nc.scalar.activation(t2a[:, hs], a2a[:], AF.Tanh,
                                         bias=c_b2a[:, 0:1])
                    nc.scalar.activation(t2b[:, hs], a2b[:], AF.Tanh,
                                         bias=c_b2b[:, 0:1])

                # ---- pair-wide P = T2*vrep (DVE, FD=1024) ----
                pa = sb.tile([128, 2 * b], BF16, tag="Pa")
                pb = sb.tile([128, 2 * b], BF16, tag="Pb")
                nc.vector.tensor_mul(pa[:], t2a[:], vt[:])
                nc.vector.tensor_mul(pb[:], t2b[:], vt[:])

                # ---- R = mblk @ P -> Rc (per half), pair-wide S/Q/M ----
                for i in range(2):
                    hs = bass.ts(i, b)
                    ra = ps.tile([128, b], F32, tag="ra")
                    rb = ps.tile([128, b], F32, tag="rb")
                    nc.tensor.matmul(ra[:], c_mblk[:], pa[:, hs],
                                     start=True, stop=True)
                    nc.tensor.matmul(rb[:], c_mblk[:], pb[:, hs],
                                     start=True, stop=True)
                    nc.scalar.copy(rca[:, hs], ra[:])
                    nc.scalar.copy(rcb[:, hs], rb[:])

                sa = sb.tile([128, 2 * b], BF16, tag="Sa")
                sbt = sb.tile([128, 2 * b], BF16, tag="Sb")
                nc.vector.tensor_mul(sa[:], rca[:], t2a[:])
                nc.vector.tensor_mul(sbt[:], rcb[:], t2b[:])
                qa = sb.tile([128, 2 * b], BF16, tag="Qa")
                qb = sb.tile([128, 2 * b], BF16, tag="Qb")
                nc.vector.tensor_mul(qa[:], rca[:], vt[:])
                nc.vector.tensor_mul(qb[:], rcb[:], vt[:])
                ma = sb.tile([128, 2 * b], BF16, tag="Ma")
                mb = sb.tile([128, 2 * b], BF16, tag="Mb")
                nc.gpsimd.tensor_mul(ma[:], sa[:], pa[:])
                nc.gpsimd.tensor_mul(mb[:], sbt[:], pb[:])
                h1sq = sb.tile([128, 2 * b], BF16, tag="H1sq")
                nc.gpsimd.tensor_mul(h1sq[:], h1[:], h1[:])

                # ---- backward per 512-half ----
                outs = sb.tile([128, 2 * b], BF16, tag="OUTS")
                for i in range(2):
                    hs = bass.ts(i, b)
                    outq = ps.tile([128, b], F32, tag="outq", bufs=2)
                    nc.tensor.matmul(outq[64:128, :], c_msum[:], sa[:, hs],
                                     start=True, stop=False)
                    nc.tensor.matmul(outq[64:128, :], c_msum[:], sbt[:, hs],
                                     start=False, stop=True)
                    dh1 = ps.tile([128, b], F32, tag="dh1")
                    nc.tensor.matmul(dh1[:], c_w2pa[:], qa[:, hs],
                                     start=True, stop=False)
                    nc.tensor.matmul(dh1[:], c_w2pb[:], qb[:, hs],
                                     start=False, stop=False)
                    nc.tensor.matmul(dh1[:], c_w2ma[:], ma[:, hs],
                                     start=False, stop=False)
                    nc.tensor.matmul(dh1[:], c_w2mb[:], mb[:, hs],
                                     start=False, stop=True)
                    da1 = sb.tile([128, b], BF16, tag="dA1")
                    nc.vector.scalar_tensor_tensor(
                        da1[:], h1sq[:, hs], 1.0, dh1[:],
                        ALU.subtract, ALU.mult)
                    nc.tensor.matmul(outq[0:64, :], c_w1n[:], da1[:],
                                     start=True, stop=True)
                    nc.vector.tensor_copy(outs[:, hs], outq[:])

                nc.sync.dma_start(outp[:, bass.ts(k, 2 * b)], outs[:])

    nc.finalize()
    return nc


def make_consts(W1, b1, W2, b2):
    """Host-side constant preparation (permutes W2 rows, folds signs)."""
    bf = ml_dtypes.bfloat16
    W1 = np.asarray(W1, np.float32)
    b1 = np.asarray(b1, np.float32)
    W2 = np.asarray(W2, np.float32)
    b2 = np.asarray(b2, np.float32)
    perm = np.empty(RANK * D, np.int64)
    for j in range(RANK):
        for i in range(D):
            perm[j * D + i] = i * RANK + j
    W2r = W2[perm, :]
    b2r = b2[perm]
    mblk = np.zeros((128, 128), np.float32)
    mblk[:64, :64] = 2.0
    mblk[64:, 64:] = 2.0
    msum = np.zeros((128, 64), np.float32)
    for i in range(64):
        msum[i, i] = -1.0
        msum[64 + i, i] = -1.0
    return {
        "w1t": np.ascontiguousarray(W1.T).astype(bf),
        "w2ta": np.ascontiguousarray(W2r[:128].T).astype(bf),
        "w2tb": np.ascontiguousarray(W2r[128:].T).astype(bf),
        "w2pa": np.ascontiguousarray(W2r[:128]).astype(bf),
        "w2pb": np.ascontiguousarray(W2r[128:]).astype(bf),
        "w2ma": np.ascontiguousarray(-W2r[:128]).astype(bf),
        "w2mb": np.ascontiguousarray(-W2r[128:]).astype(bf),
        "w1n": np.ascontiguousarray(-W1).astype(bf),
        "mblk": mblk.astype(bf),
        "msum": msum.astype(bf),
        "b1": b1.reshape(128, 1).astype(np.float32),
        "b2a": b2r[:128].reshape(128, 1).astype(np.float32),
        "b2b": b2r[128:].reshape(128, 1).astype(np.float32),
    }


_NC_CACHE = {}


def _get_program(n_rows, b):
    key = (n_rows, b)
    if key not in _NC_CACHE:
        _NC_CACHE[key] = build_program(n_rows, b)
    return _NC_CACHE[key]


def make_in_maps(inputs):
    input_ = np.asarray(inputs["input_"], np.float32)
    n = input_.shape[0]
    n_rows = n // N_CORES
    consts = make_consts(inputs["W1"], inputs["b1"], inputs["W2"], inputs["b2"])
    bfl = ml_dtypes.bfloat16
    in_maps = []
    for c in range(N_CORES):
        sh = input_[c * n_rows:(c + 1) * n_rows]          # [n_rows, 128]
        m = {"inp": np.ascontiguousarray(sh.T).astype(bfl)}  # [128, n_rows]
        m.update(consts)
        in_maps.append(m)
    return in_maps


def kernel(t, input_, W1, b1, W2, b2):
    input_ = np.asarray(input_, np.float32)
    n = input_.shape[0]
    n_rows = n // N_CORES
    nc = _get_program(n_rows, B)
    in_maps = make_in_maps(
        {"input_": input_, "W1": W1, "b1": b1, "W2": W2, "b2": b2})
    res = run_bass_kernel_spmd(nc, in_maps, list(range(N_CORES)))
    out = np.concatenate(
        [np.asarray(res.results[c]["out"]).astype(np.float32).T
         for c in range(N_CORES)], axis=0)
    return out
